# revision 15
# baseline (speedup 1.0000x reference)
"""Trainium2 Bass kernel for DecomposingAttnProcessor (pooled component softmax
cross-attention), sharded over 8 NeuronCores along the batch-component axis.

Math (per batch-component bc = c*B + b):
    q = x @ Wq ; k = enc @ Wk ; v = enc @ Wv           (per-head, dh = 64)
    scores = (q k^T) * dh^-0.5                          [H, S, E]
    pooled = mean_E scores ; wp = softmax_c(pooled)     (couples components)
    w = softmax_E(scores) * wp
    out = (w v) @ Wo + bo + x

V4 design: core i owns bc = i (full S = 4096).  The only cross-component
coupling is sum_c exp(pooled) -- a [16, 512] f32 AllReduce per 512-row
s-chunk across the 4 cores sharing the same b (groups {0,2,4,6} / {1,3,5,7}),
pipelined behind the next chunk's compute.  Everything else is local:
  - dh^-0.5 folded into kT at the encoder stage (done once per core now).
  - scoresT[e, s] per head; AV head-pairs share a PSUM bank at bases 0/64 ->
    one [128, 512] eviction per pair.
  - softmax denominators emitted as a stacked [16, 512] PSUM block via
    zero-padded ones-column matmuls (lhsT = Z[:, h:16], col 15 ones): den of
    head h lands on partition 15-h.  Pooled at rows 0:16 and den at rows
    32:48 of one shared bank.
  - coef = exp(pooled/E) / allreduce_sum / den on [16, 512] tiles; broadcast
    across partitions via a PE selector matmul into PSUM; DVE multiplies ao
    in place with in2 = PSUM (no DRAM bounce).
"""

import sys
from contextlib import ExitStack

sys.path.insert(0, "/opt/trn_rl_repo")

import numpy as np

import concourse.bass as bass  # noqa: E402
from concourse import bacc, mybir  # noqa: E402
from concourse.bass_utils import run_bass_kernel_spmd  # noqa: E402
from concourse.masks import make_identity  # noqa: E402
from concourse.tile import TileContext  # noqa: E402

# Problem dims (hardcoded per spec)
BC, S, D, E, H, C = 8, 4096, 1024, 160, 16, 4
B = BC // C  # 2
DH = D // H  # 64
SCALE = DH**-0.5  # 0.125
N_CORES = 8
E0, E1 = 128, E - 128  # encoder-token chunks (128 + 32)
ND = D // 128  # 8 chunks of the hidden dim
SL = 512  # s-chunk rows per iteration
NSC = S // SL  # 8 chunks
REPLICA_GROUPS = [[0, 2, 4, 6], [1, 3, 5, 7]]  # cores sharing the same b

F32 = mybir.dt.float32
BF16 = mybir.dt.bfloat16
EXP = mybir.ActivationFunctionType.Exp
COPY = mybir.ActivationFunctionType.Copy


def build_body(ctx, tc, d):
    nc = tc.nc
    ctx.enter_context(
        nc.allow_low_precision(reason="bf16 stats are within the 2e-2 rel-err budget")
    )
    P = 128

    pools = {}

    def pool(name, bufs, space="SBUF"):
        if name not in pools:
            pools[name] = ctx.enter_context(tc.tile_pool(name=name, bufs=bufs, space=space))
        return pools[name]

    const = pool("const", 1)
    wres = pool("wres", 1)    # Wq / Wo resident bf16
    kv_p = pool("kv", 1)      # kt / v0 / v1 / ksb (one bc)
    enc_p = pool("enc", 1)
    enct_p = pool("enct", 1)
    xin_p = pool("xin", 2)
    xt_p = pool("xt", 2)
    qt_p = pool("qt", 2)
    wa_p = pool("wa", 2)
    wb_p = pool("wb", 2)
    ao_p = pool("ao", 3)      # also hosts Wk/Wv during the encoder phase
    st_p = pool("st", 2)
    xr_p = pool("xr", 2)
    oh_p = pool("oh", 2)
    dram = pool("dram", 1, space="DRAM")

    # PSUM: 8 banks
    psA = pool("psA", 2, space="PSUM")    # E0 scores / kT-proj
    psEAV = pool("psEAV", 3, space="PSUM")  # E1 pairs + AV pairs / v-proj
    psO = pool("psO", 2, space="PSUM")    # xT transposes / Q-proj / cb / O-proj
    psPD = pool("psPD", 1, space="PSUM")  # pooled rows 0:16, den rows 32:48

    # ---- constants ----
    ident = const.tile([P, P], BF16, tag="ident")
    make_identity(nc, ident)
    ones1 = const.tile([1, P], BF16, tag="ones1")
    nc.vector.memset(ones1, 1.0)
    bo_bf = const.tile([1, D], BF16, tag="bo_bf")
    nc.gpsimd.dma_start(out=bo_bf, in_=d["bo"])  # f32 -> bf16 cast DMA
    # Z: ones at col 15 only; lhsT = Z[rows, h:16] puts a ones-column at out
    # partition 15-h with zeros accumulated above it (den stack).
    zden = const.tile([P, 16], BF16, tag="zden")
    nc.vector.memset(zden, 0.0)
    nc.vector.memset(zden[:, 15:16], 1.0)
    # sel[j]: [16, 128] selector: cb[p, s] = coef[15 - (2j + (p>=64)), s].
    # Built via PE transpose (engines cannot write rows at unaligned
    # partitions).
    sel = []
    for j in range(ND):
        selt = const.tile([P, 16], BF16, tag=f"selt{j}", name=f"selt{j}")
        nc.vector.memset(selt, 0.0)
        nc.vector.memset(selt[0:64, 15 - 2 * j : 16 - 2 * j], 1.0)
        nc.vector.memset(selt[64:128, 14 - 2 * j : 15 - 2 * j], 1.0)
        pss = psO.tile([P, 512], F32, tag="ps", name="pss").bitcast(BF16)
        nc.tensor.transpose(pss[0:16, 0:P], selt, ident)
        t = const.tile([16, P], BF16, tag=f"sel{j}", name=f"sel{j}")
        nc.scalar.activation(t, pss[0:16, 0:P], COPY)
        sel.append(t)

    # ---- inputs: enc first (PE starts on it), weights behind it ----
    en0 = enc_p.tile([P, D], BF16, tag="en0")
    en1 = enc_p.tile([E1, D], BF16, tag="en1")
    nc.gpsimd.dma_start(out=en0, in_=d["enc"][0:E0, :])
    nc.gpsimd.dma_start(out=en1, in_=d["enc"][E0:E, :])
    wq = wres.tile([P, ND * D], BF16, tag="wq")
    wo = wres.tile([P, ND * D], BF16, tag="wo")
    wk_lo = ao_p.tile([P, ND * SL], BF16, tag="ao", name="wk_lo")
    wk_hi = ao_p.tile([P, ND * SL], BF16, tag="ao", name="wk_hi")
    wv_lo = ao_p.tile([P, ND * SL], BF16, tag="ao", name="wv_lo")
    wv_hi = ao_p.tile([P, ND * SL], BF16, tag="ao", name="wv_hi")
    nc.gpsimd.dma_start(out=wk_lo, in_=d["Wk"].rearrange("(n p) d -> p n d", p=P)[:, 0:4, :])
    nc.gpsimd.dma_start(out=wk_hi, in_=d["Wk"].rearrange("(n p) d -> p n d", p=P)[:, 4:8, :])
    nc.gpsimd.dma_start(out=wv_lo, in_=d["Wv"].rearrange("(n p) d -> p n d", p=P)[:, 0:4, :])
    nc.gpsimd.dma_start(out=wv_hi, in_=d["Wv"].rearrange("(n p) d -> p n d", p=P)[:, 4:8, :])
    nc.gpsimd.dma_start(out=wq, in_=d["Wq"].rearrange("(n p) d -> p n d", p=P))
    nc.gpsimd.dma_start(out=wo, in_=d["Wo"].rearrange("(n p) d -> p n d", p=P))

    def wslice(lo, hi, i, c0, c1):
        t = lo if i < 4 else hi
        return t[:, D * (i % 4) + c0 : D * (i % 4) + c1]

    # ---- encoder phase (this core's bc only) ----
    enct = []
    for i in range(ND):
        pst = psO.tile([P, 512], F32, tag="ps", name="pst").bitcast(BF16)[:, 0:E]
        sl = slice(128 * i, 128 * (i + 1))
        nc.tensor.transpose(pst[:, 0:E0], en0[:, sl], ident)
        nc.tensor.transpose(pst[:, E0:E], en1[:, sl], ident[0:E1, 0:E1])
        t = enct_p.tile([P, E], BF16, tag=f"e{i}", name=f"e{i}")
        nc.scalar.activation(t, pst, COPY)
        enct.append(t)

    kt = kv_p.tile([P, ND * E], BF16, tag="kt")
    ksb = {}
    for j in range(ND):
        ps = psA.tile([P, 512], F32, tag="ps", name="psk")
        for i in range(ND):
            nc.tensor.matmul(
                ps[:, 0:E],
                lhsT=wslice(wk_lo, wk_hi, i, 128 * j, 128 * (j + 1)),
                rhs=enct[i],
                start=(i == 0),
                stop=(i == ND - 1),
            )
        ksl = kt[:, E * j : E * (j + 1)]
        nc.scalar.activation(ksl, ps[:, 0:E], COPY, scale=SCALE)
        kb = kv_p.tile([P, 16], BF16, tag=f"ksb{j}", name=f"ksb{j}")
        nc.gpsimd.memset(kb, 0.0)
        # head 2j ksum -> col 15-2j (rows 0:64); head 2j+1 -> col 14-2j
        nc.vector.tensor_reduce(
            kb[0:64, 15 - 2 * j : 16 - 2 * j], ksl[0:64, :],
            axis=mybir.AxisListType.X, op=mybir.AluOpType.add,
        )
        nc.vector.tensor_reduce(
            kb[64:128, 14 - 2 * j : 15 - 2 * j], ksl[64:128, :],
            axis=mybir.AxisListType.X, op=mybir.AluOpType.add,
        )
        ksb[j] = kb

    v0 = kv_p.tile([P, D], BF16, tag="v0")
    v1 = kv_p.tile([P, D], BF16, tag="v1")
    for half in range(2):
        cols = slice(512 * half, 512 * (half + 1))
        ps0 = psEAV.tile([P, 512], F32, tag="ps", name="psv0")
        ps1 = psEAV.tile([P, 512], F32, tag="ps", name="psv1")
        for i in range(ND):
            nc.tensor.matmul(
                ps0, lhsT=enct[i][:, 0:E0], rhs=wslice(wv_lo, wv_hi, i, 512 * half, 512 * (half + 1)),
                start=(i == 0), stop=(i == ND - 1),
            )
        for i in range(ND):
            nc.tensor.matmul(
                ps1[0:E1, :], lhsT=enct[i][:, E0:E], rhs=wslice(wv_lo, wv_hi, i, 512 * half, 512 * (half + 1)),
                start=(i == 0), stop=(i == ND - 1),
            )
        nc.scalar.activation(v0[:, cols], ps0, COPY)
        # replicate v1 rows at partition bases 0 and 64
        nc.scalar.activation(v1[0:E1, cols], ps1[0:E1, :], COPY)
        nc.vector.tensor_copy(v1[64 : 64 + E1, cols], ps1[0:E1, :])

    # ---- main loop over s-chunks ----
    def emit_A(sc):
        """x load + transpose + Q-projection for one 512-row s-chunk."""
        r0 = SL * sc
        xin = xin_p.tile([P, 4 * D], BF16, tag="xin")
        nc.gpsimd.dma_start(
            out=xin, in_=d["x"][r0 : r0 + SL, :].rearrange("(k p) d -> p k d", p=P)
        )
        xt = []
        for i in range(ND):
            pst = psO.tile([P, 512], F32, tag="ps", name="pst").bitcast(BF16)[:, 0:SL]
            for k in range(4):
                nc.tensor.transpose(
                    pst[:, 128 * k : 128 * (k + 1)],
                    xin[:, D * k + 128 * i : D * k + 128 * (i + 1)],
                    ident,
                )
            t = xt_p.tile([P, SL], BF16, tag=f"xt{i}", name=f"xt{i}_{sc}")
            nc.scalar.activation(t, pst, COPY)
            xt.append(t)
        qt = qt_p.tile([P, ND * SL], BF16, tag="qt")
        for j in range(ND):
            ps = psO.tile([P, 512], F32, tag="ps", name="psq")
            for i in range(ND):
                nc.tensor.matmul(
                    ps,
                    lhsT=wq[:, D * i + 128 * j : D * i + 128 * (j + 1)],
                    rhs=xt[i],
                    start=(i == 0),
                    stop=(i == ND - 1),
                )
            nc.scalar.activation(qt[:, SL * j : SL * (j + 1)], ps, COPY)
        return qt

    def emit_B(sc, qt, ao):
        """Scores + exp + AV + pooled-mm + den-mm + coef for one chunk.

        The component-sum AllReduce for this chunk is issued here; its
        consumers pipeline behind the next chunk's compute.
        """
        pd = psPD.tile([P, 512], F32, tag="ps", name=f"pd{sc}")
        for j in range(ND):
            nc.tensor.matmul(
                pd[0:16, :],
                lhsT=ksb[j],
                rhs=qt[:, SL * j : SL * (j + 1)],
                start=(j == 0),
                stop=(j == ND - 1),
                skip_group_check=True,
            )
        for j in range(ND):  # head pairs (2j, 2j+1)
            psb = psEAV.tile([P, 512], F32, tag="ps", name="psb")
            was = []
            for hp in range(2):
                h = 2 * j + hp
                hr = 64 * hp
                qsl = qt[hr : hr + 64, SL * j : SL * (j + 1)]
                ps_a = psA.tile([P, 512], F32, tag="ps", name="ps_a")
                nc.tensor.matmul(
                    ps_a, lhsT=kt[hr : hr + 64, E * j : E * j + E0], rhs=qsl,
                    start=True, stop=True,
                )
                nc.tensor.matmul(
                    psb[64 * hp : 64 * hp + E1, :],
                    lhsT=kt[hr : hr + 64, E * j + E0 : E * j + E],
                    rhs=qsl,
                    start=True, stop=True, skip_group_check=True,
                )
                wa = wa_p.tile([P, SL], BF16, tag=f"wa{hp}", name=f"wa{hp}")
                nc.scalar.activation(wa, ps_a, EXP)
                was.append(wa)
            wb = wb_p.tile([P, SL], BF16, tag="wb", name="wb")
            nc.scalar.activation(wb[0 : 64 + E1, :], psb[0 : 64 + E1, :], EXP)
            ps_av = psEAV.tile([P, 512], F32, tag="ps", name="ps_av")
            for hp in range(2):
                h = 2 * j + hp
                hr = 64 * hp
                wa = was[hp]
                wbs = wb[64 * hp : 64 * hp + E1, :]
                vsl = slice(64 * h, 64 * (h + 1))
                nc.tensor.matmul(
                    ps_av[hr : hr + 64, :], lhsT=v0[:, vsl], rhs=wa,
                    start=True, stop=False, skip_group_check=True,
                )
                nc.tensor.matmul(
                    ps_av[hr : hr + 64, :],
                    lhsT=v1[64 * hp : 64 * hp + E1, vsl],
                    rhs=wbs,
                    start=False, stop=True, skip_group_check=True,
                )
                # denominator stack: den_h -> partition 32 + 15-h
                nc.tensor.matmul(
                    pd[32 : 48 - h, :],
                    lhsT=zden[:, h:16], rhs=wa,
                    start=(h == 0), stop=False,
                    skip_group_check=True,
                    tile_position=(0, 32),
                )
                nc.tensor.matmul(
                    pd[32 : 48 - h, :],
                    lhsT=zden[64 * hp : 64 * hp + E1, h:16],
                    rhs=wbs,
                    start=False, stop=(h == H - 1),
                    skip_group_check=True,
                    tile_position=(64 * hp, 32),
                )
            nc.vector.tensor_copy(ao[:, SL * j : SL * (j + 1)], ps_av)

        # local stats + cross-component AllReduce of exp(pooled/E)
        ep = st_p.tile([16, SL], F32, tag="ep", name=f"ep{sc}")
        rd = st_p.tile([16, SL], BF16, tag="rd", name=f"rd{sc}")
        nc.scalar.activation(ep, pd[0:16, :], EXP, scale=1.0 / E)
        nc.vector.reciprocal(rd, pd[32:48, :])
        ep_d = dram.tile([16, SL], F32, tag="ep_d", name=f"ep_d{sc}", bufs=2)
        es_d = dram.tile([16, SL], F32, tag="es_d", name=f"es_d{sc}", bufs=2)
        nc.sync.dma_start(out=ep_d, in_=ep)
        nc.gpsimd.collective_compute(
            "AllReduce",
            mybir.AluOpType.add,
            replica_groups=REPLICA_GROUPS,
            ins=[ep_d[:, :]],
            outs=[es_d[:, :]],
        )
        es = st_p.tile([16, SL], F32, tag="es", name=f"es{sc}")
        nc.sync.dma_start(out=es, in_=es_d)
        rs = st_p.tile([16, SL], BF16, tag="rs", name=f"rs{sc}")
        nc.vector.reciprocal(rs, es)
        cf = st_p.tile([16, SL], BF16, tag="cf", name=f"cf{sc}")
        nc.vector.tensor_mul(rd, rd, rs)
        nc.vector.tensor_mul(cf, ep, rd)
        return cf

    def emit_D(ao, cf):
        """ao *= broadcast(coef): PE selector matmul + DVE mul (in2 = PSUM)."""
        for j in range(ND):
            cb = psO.tile([P, 512], F32, tag="ps", name="cb")
            nc.tensor.matmul(cb, lhsT=sel[j], rhs=cf, start=True, stop=True)
            sl_ao = ao[:, SL * j : SL * (j + 1)]
            nc.vector.tensor_mul(sl_ao, sl_ao, cb)

    def emit_E(sc, ao):
        """O-projection + bias + residual + store."""
        for m in range(4):
            rows = slice(SL * sc + 128 * m, SL * sc + 128 * (m + 1))
            xr = xr_p.tile([P, D], F32, tag="xr")
            nc.sync.dma_start(out=xr, in_=d["x"][rows, :])
            oh = oh_p.tile([P, D], F32, tag="oh")
            pss = [psO.tile([P, 512], F32, tag="ps", name=f"pso{hf}") for hf in range(2)]
            for half in range(2):
                nc.tensor.matmul(
                    pss[half], lhsT=ones1, rhs=bo_bf[:, 512 * half : 512 * (half + 1)],
                    start=True, stop=False, skip_group_check=True,
                )
            for i in range(ND):
                lhsT = ao[:, SL * i + 128 * m : SL * i + 128 * (m + 1)]
                for half in range(2):
                    nc.tensor.matmul(
                        pss[half],
                        lhsT=lhsT,
                        rhs=wo[:, D * i + 512 * half : D * i + 512 * (half + 1)],
                        start=False,
                        stop=(i == ND - 1),
                        skip_group_check=True,
                    )
            for half in range(2):
                cols = slice(512 * half, 512 * (half + 1))
                nc.vector.tensor_add(oh[:, cols], pss[half], xr[:, cols])
            nc.sync.dma_start(out=d["out"][rows, :], in_=oh)

    pend = None
    for sc in range(NSC):
        qt = emit_A(sc)
        ao = ao_p.tile([P, ND * SL], BF16, tag="ao", name=f"ao{sc}")
        cf = emit_B(sc, qt, ao)
        if pend is not None:
            emit_D(pend[1], pend[2])
            emit_E(pend[0], pend[1])
        pend = (sc, ao, cf)
    emit_D(pend[1], pend[2])
    emit_E(pend[0], pend[1])


def build_program(n_cores=N_CORES):
    nc = bacc.Bacc(trn_type="TRN2", target_bir_lowering=False, debug=False, num_devices=n_cores)
    d = {
        "x": nc.dram_tensor("x", [S, D], F32, kind="ExternalInput").ap(),
        "enc": nc.dram_tensor("enc", [E, D], F32, kind="ExternalInput").ap(),
        "Wq": nc.dram_tensor("Wq", [D, D], F32, kind="ExternalInput").ap(),
        "Wk": nc.dram_tensor("Wk", [D, D], F32, kind="ExternalInput").ap(),
        "Wv": nc.dram_tensor("Wv", [D, D], F32, kind="ExternalInput").ap(),
        "Wo": nc.dram_tensor("Wo", [D, D], F32, kind="ExternalInput").ap(),
        "bo": nc.dram_tensor("bo", [1, D], F32, kind="ExternalInput").ap(),
        "out": nc.dram_tensor("out", [S, D], F32, kind="ExternalOutput").ap(),
    }
    with TileContext(nc, trace_sim=False) as tc, ExitStack() as ctx:
        build_body(ctx, tc, d)
    nc.compile()
    return nc


def make_in_maps(hidden_states, encoder_hidden_states, Wq, Wk, Wv, Wo, bo, n_cores=N_CORES):
    common = {
        "Wq": np.ascontiguousarray(Wq, dtype=np.float32),
        "Wk": np.ascontiguousarray(Wk, dtype=np.float32),
        "Wv": np.ascontiguousarray(Wv, dtype=np.float32),
        "Wo": np.ascontiguousarray(Wo, dtype=np.float32),
        "bo": np.ascontiguousarray(bo, dtype=np.float32).reshape(1, D),
    }
    return [
        {
            "x": np.ascontiguousarray(hidden_states[i], dtype=np.float32),
            "enc": np.ascontiguousarray(encoder_hidden_states[i], dtype=np.float32),
            **common,
        }
        for i in range(n_cores)
    ]


def assemble(results, n_cores=N_CORES):
    return np.ascontiguousarray(
        np.stack([results[i]["out"] for i in range(n_cores)], axis=0), dtype=np.float32
    )


_NC = None


def kernel(hidden_states, encoder_hidden_states, Wq, Wk, Wv, Wo, bo):
    global _NC
    if _NC is None:
        _NC = build_program()
    in_maps = make_in_maps(hidden_states, encoder_hidden_states, Wq, Wk, Wv, Wo, bo)
    res = run_bass_kernel_spmd(_NC, in_maps, list(range(N_CORES))).results
    return assemble(res)


if __name__ == "__main__":
    build_program()
    print("compile OK")


# revision 16
# speedup vs baseline: 1.0540x; 1.0540x over previous
"""Trainium2 Bass kernel for DecomposingAttnProcessor (pooled component softmax
cross-attention), sharded over 8 NeuronCores along the batch-component axis.

Math (per batch-component bc = c*B + b):
    q = x @ Wq ; k = enc @ Wk ; v = enc @ Wv           (per-head, dh = 64)
    scores = (q k^T) * dh^-0.5                          [H, S, E]
    pooled = mean_E scores ; wp = softmax_c(pooled)     (couples components)
    w = softmax_E(scores) * wp
    out = (w v) @ Wo + bo + x

V4 design: core i owns bc = i (full S = 4096).  The only cross-component
coupling is sum_c exp(pooled) -- a [16, 512] f32 AllReduce per 512-row
s-chunk across the 4 cores sharing the same b (groups {0,2,4,6} / {1,3,5,7}),
pipelined behind the next chunk's compute.  Everything else is local:
  - dh^-0.5 folded into kT at the encoder stage (done once per core now).
  - scoresT[e, s] per head; AV head-pairs share a PSUM bank at bases 0/64 ->
    one [128, 512] eviction per pair.
  - softmax denominators emitted as a stacked [16, 512] PSUM block via
    zero-padded ones-column matmuls (lhsT = Z[:, h:16], col 15 ones): den of
    head h lands on partition 15-h.  Pooled at rows 0:16 and den at rows
    32:48 of one shared bank.
  - coef = exp(pooled/E) / allreduce_sum / den on [16, 512] tiles; broadcast
    across partitions via a PE selector matmul into PSUM; DVE multiplies ao
    in place with in2 = PSUM (no DRAM bounce).
"""

import sys
from contextlib import ExitStack

sys.path.insert(0, "/opt/trn_rl_repo")

import numpy as np

import concourse.bass as bass  # noqa: E402
from concourse import bacc, mybir  # noqa: E402
from concourse.bass_utils import run_bass_kernel_spmd  # noqa: E402
from concourse.masks import make_identity  # noqa: E402
from concourse.tile import TileContext  # noqa: E402

# Problem dims (hardcoded per spec)
BC, S, D, E, H, C = 8, 4096, 1024, 160, 16, 4
B = BC // C  # 2
DH = D // H  # 64
SCALE = DH**-0.5  # 0.125
N_CORES = 8
E0, E1 = 128, E - 128  # encoder-token chunks (128 + 32)
ND = D // 128  # 8 chunks of the hidden dim
SL = 512  # s-chunk rows per iteration
NSC = S // SL  # 8 chunks
REPLICA_GROUPS = [[0, 2, 4, 6], [1, 3, 5, 7]]  # cores sharing the same b

F32 = mybir.dt.float32
BF16 = mybir.dt.bfloat16
EXP = mybir.ActivationFunctionType.Exp
COPY = mybir.ActivationFunctionType.Copy


def build_body(ctx, tc, d):
    nc = tc.nc
    ctx.enter_context(
        nc.allow_low_precision(reason="bf16 stats are within the 2e-2 rel-err budget")
    )
    P = 128

    pools = {}

    def pool(name, bufs, space="SBUF"):
        if name not in pools:
            pools[name] = ctx.enter_context(tc.tile_pool(name=name, bufs=bufs, space=space))
        return pools[name]

    const = pool("const", 1)
    wres = pool("wres", 1)    # Wq / Wo resident bf16
    kv_p = pool("kv", 1)      # kt / v0 / v1 / ksb (one bc)
    enc_p = pool("enc", 1)
    enct_p = pool("enct", 1)
    xin_p = pool("xin", 2)
    xt_p = pool("xt", 2)
    qt_p = pool("qt", 2)
    wa_p = pool("wa", 2)
    wb_p = pool("wb", 2)
    ao_p = pool("ao", 3)      # also hosts Wk/Wv during the encoder phase
    st_p = pool("st", 2)
    xr_p = pool("xr", 2)
    oh_p = pool("oh", 2)
    dram = pool("dram", 1, space="DRAM")

    # PSUM: 8 banks
    psA = pool("psA", 2, space="PSUM")    # E0 scores / kT-proj
    psEAV = pool("psEAV", 3, space="PSUM")  # E1 pairs + AV pairs / v-proj
    psO = pool("psO", 2, space="PSUM")    # xT transposes / Q-proj / cb / O-proj
    psPD = pool("psPD", 1, space="PSUM")  # pooled rows 0:16, den rows 32:48

    # ---- constants ----
    ident = const.tile([P, P], BF16, tag="ident")
    make_identity(nc, ident)
    ones1 = const.tile([1, P], BF16, tag="ones1")
    nc.vector.memset(ones1, 1.0)
    bo_bf = const.tile([1, D], BF16, tag="bo_bf")
    nc.gpsimd.dma_start(out=bo_bf, in_=d["bo"])  # f32 -> bf16 cast DMA
    # Z: ones at col 15 only; lhsT = Z[rows, h:16] puts a ones-column at out
    # partition 15-h with zeros accumulated above it (den stack).
    zden = const.tile([P, 16], BF16, tag="zden")
    nc.vector.memset(zden, 0.0)
    nc.vector.memset(zden[:, 15:16], 1.0)
    # sel[j]: [16, 128] selector: cb[p, s] = coef[15 - (2j + (p>=64)), s].
    # Built via PE transpose (engines cannot write rows at unaligned
    # partitions).
    sel = []
    for j in range(ND):
        selt = const.tile([P, 16], BF16, tag=f"selt{j}", name=f"selt{j}")
        nc.vector.memset(selt, 0.0)
        nc.vector.memset(selt[0:64, 15 - 2 * j : 16 - 2 * j], 1.0)
        nc.vector.memset(selt[64:128, 14 - 2 * j : 15 - 2 * j], 1.0)
        pss = psO.tile([P, 512], F32, tag="ps", name="pss").bitcast(BF16)
        nc.tensor.transpose(pss[0:16, 0:P], selt, ident)
        t = const.tile([16, P], BF16, tag=f"sel{j}", name=f"sel{j}")
        nc.scalar.activation(t, pss[0:16, 0:P], COPY)
        sel.append(t)

    # ---- inputs: enc first (PE starts on it), weights behind it ----
    en0 = enc_p.tile([P, D], BF16, tag="en0")
    en1 = enc_p.tile([E1, D], BF16, tag="en1")
    nc.gpsimd.dma_start(out=en0, in_=d["enc"][0:E0, :])
    nc.gpsimd.dma_start(out=en1, in_=d["enc"][E0:E, :])
    wq = wres.tile([P, ND * D], BF16, tag="wq")
    wo = wres.tile([P, ND * D], BF16, tag="wo")
    wk_lo = ao_p.tile([P, ND * SL], BF16, tag="ao", name="wk_lo")
    wk_hi = ao_p.tile([P, ND * SL], BF16, tag="ao", name="wk_hi")
    wv_lo = ao_p.tile([P, ND * SL], BF16, tag="ao", name="wv_lo")
    wv_hi = ao_p.tile([P, ND * SL], BF16, tag="ao", name="wv_hi")
    nc.gpsimd.dma_start(out=wk_lo, in_=d["Wk"].rearrange("(n p) d -> p n d", p=P)[:, 0:4, :])
    nc.gpsimd.dma_start(out=wk_hi, in_=d["Wk"].rearrange("(n p) d -> p n d", p=P)[:, 4:8, :])
    nc.gpsimd.dma_start(out=wv_lo, in_=d["Wv"].rearrange("(n p) d -> p n d", p=P)[:, 0:4, :])
    nc.gpsimd.dma_start(out=wv_hi, in_=d["Wv"].rearrange("(n p) d -> p n d", p=P)[:, 4:8, :])
    nc.gpsimd.dma_start(out=wq, in_=d["Wq"].rearrange("(n p) d -> p n d", p=P))
    nc.gpsimd.dma_start(out=wo, in_=d["Wo"].rearrange("(n p) d -> p n d", p=P))

    def wslice(lo, hi, i, c0, c1):
        t = lo if i < 4 else hi
        return t[:, D * (i % 4) + c0 : D * (i % 4) + c1]

    # ---- encoder phase (this core's bc only) ----
    enct = []
    for i in range(ND):
        pst = psO.tile([P, 512], F32, tag="ps", name="pst").bitcast(BF16)[:, 0:E]
        sl = slice(128 * i, 128 * (i + 1))
        nc.tensor.transpose(pst[:, 0:E0], en0[:, sl], ident)
        nc.tensor.transpose(pst[:, E0:E], en1[:, sl], ident[0:E1, 0:E1])
        t = enct_p.tile([P, E], BF16, tag=f"e{i}", name=f"e{i}")
        nc.scalar.activation(t, pst, COPY)
        enct.append(t)

    kt = kv_p.tile([P, ND * E], BF16, tag="kt")
    ksb = {}
    for j in range(ND):
        ps = psA.tile([P, 512], F32, tag="ps", name="psk")
        for i in range(ND):
            nc.tensor.matmul(
                ps[:, 0:E],
                lhsT=wslice(wk_lo, wk_hi, i, 128 * j, 128 * (j + 1)),
                rhs=enct[i],
                start=(i == 0),
                stop=(i == ND - 1),
            )
        ksl = kt[:, E * j : E * (j + 1)]
        nc.scalar.activation(ksl, ps[:, 0:E], COPY, scale=SCALE)
        kb = kv_p.tile([P, 16], BF16, tag=f"ksb{j}", name=f"ksb{j}")
        nc.gpsimd.memset(kb, 0.0)
        # head 2j ksum -> col 15-2j (rows 0:64); head 2j+1 -> col 14-2j
        nc.vector.tensor_reduce(
            kb[0:64, 15 - 2 * j : 16 - 2 * j], ksl[0:64, :],
            axis=mybir.AxisListType.X, op=mybir.AluOpType.add,
        )
        nc.vector.tensor_reduce(
            kb[64:128, 14 - 2 * j : 15 - 2 * j], ksl[64:128, :],
            axis=mybir.AxisListType.X, op=mybir.AluOpType.add,
        )
        ksb[j] = kb

    v0 = kv_p.tile([P, D], BF16, tag="v0")
    v1 = kv_p.tile([P, D], BF16, tag="v1")
    for half in range(2):
        cols = slice(512 * half, 512 * (half + 1))
        ps0 = psEAV.tile([P, 512], F32, tag="ps", name="psv0")
        ps1 = psEAV.tile([P, 512], F32, tag="ps", name="psv1")
        for i in range(ND):
            nc.tensor.matmul(
                ps0, lhsT=enct[i][:, 0:E0], rhs=wslice(wv_lo, wv_hi, i, 512 * half, 512 * (half + 1)),
                start=(i == 0), stop=(i == ND - 1),
            )
        for i in range(ND):
            nc.tensor.matmul(
                ps1[0:E1, :], lhsT=enct[i][:, E0:E], rhs=wslice(wv_lo, wv_hi, i, 512 * half, 512 * (half + 1)),
                start=(i == 0), stop=(i == ND - 1),
            )
        nc.scalar.activation(v0[:, cols], ps0, COPY)
        # replicate v1 rows at partition bases 0 and 64
        nc.scalar.activation(v1[0:E1, cols], ps1[0:E1, :], COPY)
        nc.vector.tensor_copy(v1[64 : 64 + E1, cols], ps1[0:E1, :])

    # ---- main loop over s-chunks ----
    def emit_A(sc, xin):
        """x transpose + Q-projection for one (prefetched) 512-row s-chunk."""
        xt = []
        for i in range(ND):
            pst = psO.tile([P, 512], F32, tag="ps", name="pst").bitcast(BF16)[:, 0:SL]
            for k in range(4):
                nc.tensor.transpose(
                    pst[:, 128 * k : 128 * (k + 1)],
                    xin[:, D * k + 128 * i : D * k + 128 * (i + 1)],
                    ident,
                )
            t = xt_p.tile([P, SL], BF16, tag=f"xt{i}", name=f"xt{i}_{sc}")
            nc.scalar.activation(t, pst, COPY)
            xt.append(t)
        qt = qt_p.tile([P, ND * SL], BF16, tag="qt")
        for j in range(ND):
            ps = psO.tile([P, 512], F32, tag="ps", name="psq")
            for i in range(ND):
                nc.tensor.matmul(
                    ps,
                    lhsT=wq[:, D * i + 128 * j : D * i + 128 * (j + 1)],
                    rhs=xt[i],
                    start=(i == 0),
                    stop=(i == ND - 1),
                )
            nc.scalar.activation(qt[:, SL * j : SL * (j + 1)], ps, COPY)
        return qt

    def emit_B(sc, qt, ao, filler):
        """Scores + exp + AV + pooled-mm + den-mm + coef for one chunk.

        `filler` is a list of callables (previous chunk's D/E matmul groups)
        consumed one per head-pair so the PE has independent work queued
        while it would otherwise wait on the exp evictions.
        """
        if sc + 1 < NSC:
            prefetch_xin(sc + 1)
        pd = psPD.tile([P, 512], F32, tag="ps", name=f"pd{sc}")
        for j in range(ND):
            nc.tensor.matmul(
                pd[0:16, :],
                lhsT=ksb[j],
                rhs=qt[:, SL * j : SL * (j + 1)],
                start=(j == 0),
                stop=(j == ND - 1),
                skip_group_check=True,
            )
        fi = 0
        for j in range(ND):  # head pairs (2j, 2j+1)
            psb = psEAV.tile([P, 512], F32, tag="ps", name="psb")
            was = []
            for hp in range(2):
                h = 2 * j + hp
                hr = 64 * hp
                qsl = qt[hr : hr + 64, SL * j : SL * (j + 1)]
                ps_a = psA.tile([P, 512], F32, tag="ps", name="ps_a")
                nc.tensor.matmul(
                    ps_a, lhsT=kt[hr : hr + 64, E * j : E * j + E0], rhs=qsl,
                    start=True, stop=True,
                )
                nc.tensor.matmul(
                    psb[64 * hp : 64 * hp + E1, :],
                    lhsT=kt[hr : hr + 64, E * j + E0 : E * j + E],
                    rhs=qsl,
                    start=True, stop=True, skip_group_check=True,
                )
                wa = wa_p.tile([P, SL], BF16, tag=f"wa{hp}", name=f"wa{hp}")
                nc.scalar.activation(wa, ps_a, EXP)
                was.append(wa)
            wb = wb_p.tile([P, SL], BF16, tag="wb", name="wb")
            nc.scalar.activation(wb[0 : 64 + E1, :], psb[0 : 64 + E1, :], EXP)
            # fill the exp-wait window with independent prev-chunk matmuls
            if fi < len(filler):
                filler[fi]()
                fi += 1
            ps_av = psEAV.tile([P, 512], F32, tag="ps", name="ps_av")
            for hp in range(2):
                h = 2 * j + hp
                hr = 64 * hp
                wa = was[hp]
                wbs = wb[64 * hp : 64 * hp + E1, :]
                vsl = slice(64 * h, 64 * (h + 1))
                nc.tensor.matmul(
                    ps_av[hr : hr + 64, :], lhsT=v0[:, vsl], rhs=wa,
                    start=True, stop=False, skip_group_check=True,
                )
                nc.tensor.matmul(
                    ps_av[hr : hr + 64, :],
                    lhsT=v1[64 * hp : 64 * hp + E1, vsl],
                    rhs=wbs,
                    start=False, stop=True, skip_group_check=True,
                )
                # denominator stack: den_h -> partition 32 + 15-h
                nc.tensor.matmul(
                    pd[32 : 48 - h, :],
                    lhsT=zden[:, h:16], rhs=wa,
                    start=(h == 0), stop=False,
                    skip_group_check=True,
                    tile_position=(0, 32),
                )
                nc.tensor.matmul(
                    pd[32 : 48 - h, :],
                    lhsT=zden[64 * hp : 64 * hp + E1, h:16],
                    rhs=wbs,
                    start=False, stop=(h == H - 1),
                    skip_group_check=True,
                    tile_position=(64 * hp, 32),
                )
            nc.vector.tensor_copy(ao[:, SL * j : SL * (j + 1)], ps_av)
        while fi < len(filler):
            filler[fi]()
            fi += 1

        # local stats; den copied out fast so the pd bank frees before the
        # slow DVE reciprocal runs.
        ep = st_p.tile([16, SL], F32, tag="ep", name=f"ep{sc}")
        dencp = st_p.tile([16, SL], F32, tag="dencp", name=f"dencp{sc}")
        nc.scalar.activation(ep, pd[0:16, :], EXP, scale=1.0 / E)
        nc.scalar.activation(dencp, pd[32:48, :], COPY)
        rd = st_p.tile([16, SL], BF16, tag="rd", name=f"rd{sc}")
        nc.vector.reciprocal(rd, dencp)
        ep_d = dram.tile([16, SL], F32, tag="ep_d", name=f"ep_d{sc}", bufs=2)
        es_d = dram.tile([16, SL], F32, tag="es_d", name=f"es_d{sc}", bufs=2)
        nc.sync.dma_start(out=ep_d, in_=ep)
        nc.gpsimd.collective_compute(
            "AllReduce",
            mybir.AluOpType.add,
            replica_groups=REPLICA_GROUPS,
            ins=[ep_d[:, :]],
            outs=[es_d[:, :]],
        )
        es = st_p.tile([16, SL], F32, tag="es", name=f"es{sc}")
        nc.sync.dma_start(out=es, in_=es_d)
        rs = st_p.tile([16, SL], BF16, tag="rs", name=f"rs{sc}")
        nc.vector.reciprocal(rs, es)
        cf = st_p.tile([16, SL], BF16, tag="cf", name=f"cf{sc}")
        nc.vector.tensor_mul(rd, rd, rs)
        nc.vector.tensor_mul(cf, ep, rd)
        return cf

    def de_groups(sc, ao, cf):
        """Previous chunk's D (coef apply) + E (O-proj/store) as 8 groups."""
        groups = []

        def dgroup(j0):
            def go():
                for j in (j0, j0 + 1):
                    cb = psO.tile([P, 512], F32, tag="ps", name="cb")
                    nc.tensor.matmul(cb, lhsT=sel[j], rhs=cf, start=True, stop=True)
                    sl_ao = ao[:, SL * j : SL * (j + 1)]
                    nc.vector.tensor_mul(sl_ao, sl_ao, cb)
            return go

        def egroup(m):
            def go():
                rows = slice(SL * sc + 128 * m, SL * sc + 128 * (m + 1))
                xr = xr_p.tile([P, D], F32, tag="xr", name="xr")
                nc.sync.dma_start(out=xr, in_=d["x"][rows, :])
                oh = oh_p.tile([P, D], F32, tag="oh", name="oh")
                pss = [psO.tile([P, 512], F32, tag="ps", name=f"pso{hf}") for hf in range(2)]
                for half in range(2):
                    nc.tensor.matmul(
                        pss[half], lhsT=ones1, rhs=bo_bf[:, 512 * half : 512 * (half + 1)],
                        start=True, stop=False, skip_group_check=True,
                    )
                for i in range(ND):
                    lhsT = ao[:, SL * i + 128 * m : SL * i + 128 * (m + 1)]
                    for half in range(2):
                        nc.tensor.matmul(
                            pss[half],
                            lhsT=lhsT,
                            rhs=wo[:, D * i + 512 * half : D * i + 512 * (half + 1)],
                            start=False,
                            stop=(i == ND - 1),
                            skip_group_check=True,
                        )
                for half in range(2):
                    cols = slice(512 * half, 512 * (half + 1))
                    nc.vector.tensor_add(oh[:, cols], pss[half], xr[:, cols])
                nc.sync.dma_start(out=d["out"][rows, :], in_=oh)
            return go

        for j0 in (0, 2, 4, 6):
            groups.append(dgroup(j0))
        for m in range(4):
            groups.append(egroup(m))
        return groups

    pend = None
    pend = None
    xins = {}

    def prefetch_xin(sc):
        if sc in xins:
            return
        r0 = SL * sc
        xin = xin_p.tile([P, 4 * D], BF16, tag="xin", name=f"xin{sc}")
        nc.gpsimd.dma_start(
            out=xin, in_=d["x"][r0 : r0 + SL, :].rearrange("(k p) d -> p k d", p=P)
        )
        xins[sc] = xin

    for sc in range(NSC):
        prefetch_xin(sc)
        qt = emit_A(sc, xins.pop(sc))
        ao = ao_p.tile([P, ND * SL], BF16, tag="ao", name=f"ao{sc}")
        filler = de_groups(*pend) if pend is not None else []
        cf = emit_B(sc, qt, ao, filler)
        pend = (sc, ao, cf)
    for g in de_groups(*pend):
        g()


def build_program(n_cores=N_CORES):
    nc = bacc.Bacc(trn_type="TRN2", target_bir_lowering=False, debug=False, num_devices=n_cores)
    d = {
        "x": nc.dram_tensor("x", [S, D], F32, kind="ExternalInput").ap(),
        "enc": nc.dram_tensor("enc", [E, D], F32, kind="ExternalInput").ap(),
        "Wq": nc.dram_tensor("Wq", [D, D], F32, kind="ExternalInput").ap(),
        "Wk": nc.dram_tensor("Wk", [D, D], F32, kind="ExternalInput").ap(),
        "Wv": nc.dram_tensor("Wv", [D, D], F32, kind="ExternalInput").ap(),
        "Wo": nc.dram_tensor("Wo", [D, D], F32, kind="ExternalInput").ap(),
        "bo": nc.dram_tensor("bo", [1, D], F32, kind="ExternalInput").ap(),
        "out": nc.dram_tensor("out", [S, D], F32, kind="ExternalOutput").ap(),
    }
    with TileContext(nc, trace_sim=False) as tc, ExitStack() as ctx:
        build_body(ctx, tc, d)
    nc.compile()
    return nc


def make_in_maps(hidden_states, encoder_hidden_states, Wq, Wk, Wv, Wo, bo, n_cores=N_CORES):
    common = {
        "Wq": np.ascontiguousarray(Wq, dtype=np.float32),
        "Wk": np.ascontiguousarray(Wk, dtype=np.float32),
        "Wv": np.ascontiguousarray(Wv, dtype=np.float32),
        "Wo": np.ascontiguousarray(Wo, dtype=np.float32),
        "bo": np.ascontiguousarray(bo, dtype=np.float32).reshape(1, D),
    }
    return [
        {
            "x": np.ascontiguousarray(hidden_states[i], dtype=np.float32),
            "enc": np.ascontiguousarray(encoder_hidden_states[i], dtype=np.float32),
            **common,
        }
        for i in range(n_cores)
    ]


def assemble(results, n_cores=N_CORES):
    return np.ascontiguousarray(
        np.stack([results[i]["out"] for i in range(n_cores)], axis=0), dtype=np.float32
    )


_NC = None


def kernel(hidden_states, encoder_hidden_states, Wq, Wk, Wv, Wo, bo):
    global _NC
    if _NC is None:
        _NC = build_program()
    in_maps = make_in_maps(hidden_states, encoder_hidden_states, Wq, Wk, Wv, Wo, bo)
    res = run_bass_kernel_spmd(_NC, in_maps, list(range(N_CORES))).results
    return assemble(res)


if __name__ == "__main__":
    build_program()
    print("compile OK")
